# revision 1
# baseline (speedup 1.0000x reference)
"""Trainium2 Bass kernel for two-stage retrieval-kNN (router topk -> fine rescore).

Strategy (token-sharded, no collectives):
  - 4096 tokens split across 8 cores (512 each). Every core holds full tables.
  - Router logits via fp8(e4m3) DoubleRow PE matmul (2 k-tiles per
    instruction, fp32 accumulate) streamed through 2048-wide PSUM chunks.
  - Per chunk: DVE max8 + max_index directly on PSUM fp32 -> top-8
    values + positions per 2048 columns (256 L1 slots per token).
  - One final merge per m-tile: positions + 8b-quantized values packed into
    fp32 ints; top-40 candidates via 5 max8/match_replace rounds.
  - Top-4 by packed rank kept directly; packed ranks 5-40 rescored EXACTLY
    in fp32 (x . W_router column gathers) to fix fp8 ordering noise near the
    top-20 boundary; final top-20 = 4 direct + top-16 of the rescored window.
  - Fine stage: q = x @ W_enc (fp32 PE), gather K rows (fp32), fine scores
    in fp32 (exact top-10 selection), softmax, gather V rows (fp16),
    weighted sum in fp32.
  - Two token-group passes (m-tiles {0,1} then {2,3}): W is streamed twice,
    and group 0's gather/rescore tail fills DVE bubbles under group 1's
    router stream.
"""
import sys
sys.path.insert(0, '/opt/trn_rl_repo')

import numpy as np
import ml_dtypes
from contextlib import ExitStack

import concourse.bass as bass
import concourse.mybir as mybir
import concourse.tile as tile
from concourse import bacc
from concourse.bass_utils import run_bass_kernel_spmd

F32 = mybir.dt.float32
F16 = mybir.dt.float16
F8 = mybir.dt.float8e4
U32 = mybir.dt.uint32
I32 = mybir.dt.int32
AL = mybir.AluOpType
AXX = mybir.AxisListType.X
PM = mybir.MatmulPerfMode

NEG = -1.0e30

D = 1024           # model dim
R = 128            # knowledge rank
NK = 65536         # knowledge slots
COARSE_K = 20
FINE_K = 10
CHUNK = 2048       # router chunk width (4 PSUM banks)
PAIRS = 4          # k-tile pairs (D / 256)

XSCALE = 16.0      # x fp8 quant scale
WSCALE = 1024.0    # W fp8 quant scale
LSCALE = XSCALE * WSCALE  # PSUM logits = LSCALE * true logit

# packed-candidate quantization: vq = clamp(round((v/LSCALE - VLO)*80), 0, 255)
VLO = 1.2
VSCALE = 80.0

CAND = 40          # packed candidates kept (5 max8 rounds)
DIRECT = 4         # top packed ranks kept without rescore
WIN = 36           # packed ranks DIRECT..DIRECT+WIN rescored exactly
RSUB = 8           # rescore gather sub-batch


def build(n_chunks=32, m_tiles=4, cores=8, passes=2, repeat=1):
    """Build the per-core NEFF. Token count = m_tiles*128 per core."""
    ntok = m_tiles * 128
    nk = n_chunks * CHUNK
    nslot = n_chunks * 8
    nc = bacc.Bacc("TRN2", target_bir_lowering=False, debug=False)

    xT8 = nc.dram_tensor("xT8", [D, ntok], F8, kind="ExternalInput").ap()
    xT32 = nc.dram_tensor("xT32", [D, ntok], F32, kind="ExternalInput").ap()
    x32 = nc.dram_tensor("x32", [ntok, D], F32, kind="ExternalInput").ap()
    W8L = nc.dram_tensor("W8L", [n_chunks * 128, 8 * CHUNK], F8,
                         kind="ExternalInput").ap()
    WT32 = nc.dram_tensor("WT32", [nk, D], F32, kind="ExternalInput").ap()
    K32 = nc.dram_tensor("K32", [nk, R], F32, kind="ExternalInput").ap()
    V16 = nc.dram_tensor("V16", [nk, D], F16, kind="ExternalInput").ap()
    Wenc = nc.dram_tensor("Wenc", [D, R], F32, kind="ExternalInput").ap()
    out = nc.dram_tensor("out", [ntok, D], F32, kind="ExternalOutput").ap()

    with tile.TileContext(nc) as tc, ExitStack() as ctx:
        sb = ctx.enter_context(tc.tile_pool(name="sb", bufs=1))
        wp = ctx.enter_context(tc.tile_pool(name="wp", bufs=2))
        ps = ctx.enter_context(tc.tile_pool(name="ps", bufs=2, space="PSUM"))
        gp = ctx.enter_context(tc.tile_pool(name="gp", bufs=2))

        # ---------------- constants ----------------
        # iotaM[slot] = 65535 - chunkbase(slot); slot = chunk*8 + r
        ioI = sb.tile([128, nslot], I32, tag="ioI", name="ioI")
        nc.gpsimd.iota(ioI[:].rearrange("p (g x) -> p g x", x=8),
                       pattern=[[CHUNK, n_chunks], [0, 8]], base=0,
                       channel_multiplier=0)
        iotaM = sb.tile([128, nslot], F32, tag="iotaM", name="iotaM")
        nc.vector.tensor_copy(iotaM[:], ioI[:])
        nc.vector.tensor_scalar(iotaM[:], iotaM[:], -1.0, 65535.0,
                                op0=AL.mult, op1=AL.add)
        io20 = sb.tile([128, COARSE_K], I32, tag="io20", name="io20")
        nc.gpsimd.iota(io20[:], pattern=[[1, COARSE_K]], base=0,
                       channel_multiplier=0)
        io20f = sb.tile([128, COARSE_K], F32, tag="io20f", name="io20f")
        nc.vector.tensor_copy(io20f[:], io20[:])
        ioWN = sb.tile([128, WIN], I32, tag="ioWN", name="ioWN")
        nc.gpsimd.iota(ioWN[:], pattern=[[1, WIN]], base=0, channel_multiplier=0)
        ioWNf = sb.tile([128, WIN], F32, tag="ioWNf", name="ioWNf")
        nc.vector.tensor_copy(ioWNf[:], ioWN[:])

        # ---------------- static loads ----------------
        # fp8 x pair-tiles: xt8[m][t] = [128, 2, 128]
        xt8 = [[None] * PAIRS for _ in range(m_tiles)]
        for m in range(m_tiles):
            for t in range(PAIRS):
                tl = sb.tile([128, 2, 128], F8, tag=f"xt8_{m}_{t}",
                             name=f"xt8_{m}_{t}")
                nc.sync.dma_start(
                    tl[:],
                    xT8[256 * t:256 * (t + 1), m * 128:(m + 1) * 128]
                    .rearrange("(i p) mm -> p i mm", i=2))
                xt8[m][t] = tl
        wenc = []
        for k in range(8):
            we = sb.tile([128, R], F32, tag=f"wenc_{k}", name=f"wenc_{k}")
            nc.sync.dma_start(we[:], Wenc[k * 128:(k + 1) * 128, :])
            wenc.append(we)
        xt32 = []
        for k in range(8):
            t32 = sb.tile([128, ntok], F32, tag=f"xt32_{k}", name=f"xt32_{k}")
            nc.sync.dma_start(t32[:], xT32[k * 128:(k + 1) * 128, :])
            xt32.append(t32)

        # ---------------- q = x @ W_enc (fp32 PE) ----------------
        q16 = []
        for m in range(m_tiles):
            q_ps = ps.tile([128, CHUNK], F32, tag="pl", name="qps")
            msl = slice(m * 128, (m + 1) * 128)
            for k in range(8):
                nc.tensor.matmul(q_ps[:, 0:R], xt32[k][:, msl], wenc[k][:],
                                 start=(k == 0), stop=(k == 7))
            q = sb.tile([128, R], F32, tag=f"q32_{m}", name=f"q32_{m}")
            nc.scalar.copy(q[:], q_ps[:, 0:R])
            q16.append(q)

        # per-m-tile L1 arrays
        l1v = [sb.tile([128, nslot], F32, tag=f"l1v_{m}", name=f"l1v_{m}")
               for m in range(m_tiles)]
        l1p = [sb.tile([128, nslot], U32, tag=f"l1p_{m}", name=f"l1p_{m}")
               for m in range(m_tiles)]

        cand = [sb.tile([128, CAND], F32, tag=f"cand_{m}", name=f"cand_{m}")
                for m in range(m_tiles)]
        scr = sb.tile([128, CAND + nslot], F32, tag="scr", name="scr")

        # ---------------- token-group passes ----------------
        # stream W once per group; group g's tail overlaps group g+1's
        # stream (DVE fills its bubbles with tail work).
        if isinstance(passes, (list, tuple)):
            groups, b = [], 0
            for g in passes:
                groups.append(list(range(b, b + g)))
                b += g
            assert b == m_tiles
        elif passes == 1:
            groups = [list(range(m_tiles))]
        else:
            gs = m_tiles // passes
            groups = [list(range(g * gs, (g + 1) * gs))
                      for g in range(passes)]
        def _emit_all():
            for ms in groups:
                # ---------------- fp8 router stream ----------------
                for n in range(n_chunks):
                    w8c = wp.tile([128, 8 * CHUNK], F8, tag="w8", name="w8")
                    nc.sync.dma_start(w8c[:], W8L[n * 128:(n + 1) * 128, :])
                    w8v = w8c[:].rearrange("p (t i c) -> p t i c", t=PAIRS, i=2)
                    for m in ms:
                        pl = ps.tile([128, CHUNK], F32, tag="pl", name="pl")
                        for t in range(PAIRS):
                            for cs in range(CHUNK // 512):
                                nc.tensor.matmul(
                                    pl[:, cs * 512:(cs + 1) * 512],
                                    xt8[m][t][:],
                                    w8v[:, t, :, cs * 512:(cs + 1) * 512],
                                    start=(t == 0), stop=(t == PAIRS - 1),
                                    perf_mode=PM.DoubleRow)
                        sl = slice(n * 8, (n + 1) * 8)
                        nc.vector.max(out=l1v[m][:, sl], in_=pl[:])
                        nc.vector.max_index(out=l1p[m][:, sl], in_max=l1v[m][:, sl],
                                            in_values=pl[:])

                # ---------------- single final merge per m-tile ----------------
                for m in ms:
                    nc.vector.memset(scr[:, 0:CAND], NEG)
                    blk = scr[:, CAND:CAND + nslot]
                    # vq = clamp(round((v/LSCALE - VLO)*80), 0, 255)
                    nc.vector.tensor_scalar(blk, l1v[m][:], VSCALE / LSCALE,
                                            -VLO * VSCALE, op0=AL.mult, op1=AL.add)
                    ti = sb.tile([128, nslot], I32, tag="pk_i", name="pk_i")
                    nc.vector.tensor_copy(ti[:], blk)
                    nc.vector.tensor_copy(blk, ti[:])
                    nc.vector.tensor_scalar_min(blk, blk, 255.0)
                    nc.vector.tensor_scalar_max(blk, blk, 0.0)
                    # negpos = (65535 - chunkbase) - within
                    pf = sb.tile([128, nslot], F32, tag="pk_f", name="pk_f")
                    nc.vector.tensor_copy(pf[:], l1p[m][:])
                    nc.vector.tensor_tensor(out=pf[:], in0=iotaM[:], in1=pf[:],
                                            op=AL.subtract)
                    # packed = vq*65536 + negpos
                    nc.vector.scalar_tensor_tensor(out=blk, in0=blk, scalar=65536.0,
                                                   in1=pf[:], op0=AL.mult, op1=AL.add)
                    for r in range(CAND // 8):
                        nc.vector.max(out=cand[m][:, r * 8:(r + 1) * 8], in_=scr[:])
                        if r < CAND // 8 - 1:
                            nc.vector.match_replace(
                                out=scr[:], in_to_replace=cand[m][:, r * 8:(r + 1) * 8],
                                in_values=scr[:], imm_value=NEG)

                # ---------------- per-m-tile tail ----------------
                for m in ms:
                    msl = slice(m * 128, (m + 1) * 128)
                    # unpack positions of all CAND candidates
                    cpI = sb.tile([128, CAND], I32, tag="cpI", name="cpI")
                    nc.vector.tensor_copy(cpI[:], cand[m][:])
                    nc.vector.tensor_scalar(cpI[:], cpI[:], 65535, None,
                                            op0=AL.bitwise_and)
                    cpF = sb.tile([128, CAND], F32, tag="cpF", name="cpF")
                    nc.vector.tensor_copy(cpF[:], cpI[:])
                    nc.vector.tensor_scalar(cpF[:], cpF[:], -1.0, 65535.0,
                                            op0=AL.mult, op1=AL.add)  # = positions
                    cpU = sb.tile([128, CAND], U32, tag="cpU", name="cpU")
                    nc.vector.tensor_copy(cpU[:], cpF[:])

                    # ---- exact rescore of window slots [DIRECT, DIRECT+WIN) ----
                    x32t = sb.tile([128, D], F32, tag="x32t", name="x32t")
                    nc.sync.dma_start(x32t[:], x32[msl, :])
                    vex = sb.tile([128, WIN], F32, tag="vex", name="vex")
                    junk = sb.tile([128, D], F32, tag="junk", name="junk")
                    for g0 in range(0, WIN, RSUB):
                        nb = min(RSUB, WIN - g0)
                        wc = gp.tile([128, RSUB, D], F32, tag="gat", name="wc")
                        for s in range(nb):
                            nc.gpsimd.indirect_dma_start(
                                out=wc[:, s, :], out_offset=None, in_=WT32,
                                in_offset=bass.IndirectOffsetOnAxis(
                                    ap=cpU[:, DIRECT + g0 + s:DIRECT + g0 + s + 1],
                                    axis=0))
                        for s in range(nb):
                            nc.vector.scalar_tensor_tensor(
                                out=junk[:], in0=wc[:, s, :], scalar=1.0,
                                in1=x32t[:], op0=AL.mult, op1=AL.mult,
                                accum_out=vex[:, g0 + s:g0 + s + 1])

                    # top-12 of the WIN exact window values -> window slot ids
                    vw = sb.tile([128, WIN], F32, tag="vw", name="vw")
                    nc.vector.tensor_copy(vw[:], vex[:])
                    w8a = sb.tile([128, 8], F32, tag="w8a", name="w8a")
                    nc.vector.max(out=w8a[:], in_=vw[:])
                    nc.vector.match_replace(out=vw[:], in_to_replace=w8a[:],
                                            in_values=vw[:], imm_value=NEG)
                    w8b = sb.tile([128, 8], F32, tag="w8b", name="w8b")
                    nc.vector.max(out=w8b[:], in_=vw[:])
                    wia = sb.tile([128, 8], U32, tag="wia", name="wia")
                    nc.vector.max_index(out=wia[:], in_max=w8a[:], in_values=vex[:])
                    wib = sb.tile([128, 8], U32, tag="wib", name="wib")
                    nc.vector.max_index(out=wib[:], in_max=w8b[:], in_values=vex[:])

                    # Kidx[128, 20] = cpF[0:8] | winpos(top12 exact)
                    kidxF = sb.tile([128, COARSE_K], F32, tag="kidxF", name="kidxF")
                    nc.vector.tensor_copy(kidxF[:, 0:DIRECT], cpF[:, 0:DIRECT])
                    NW = COARSE_K - DIRECT
                    wsel = sb.tile([128, NW], F32, tag="wsel", name="wsel")
                    sel16 = sb.tile([128, NW], F32, tag="sel16", name="sel16")
                    wiaf = sb.tile([128, 8], F32, tag="wiaf", name="wiaf")
                    nc.vector.tensor_copy(wiaf[:], wia[:])
                    wibf = sb.tile([128, 8], F32, tag="wibf", name="wibf")
                    nc.vector.tensor_copy(wibf[:], wib[:])
                    nc.vector.tensor_copy(sel16[:, 0:8], wiaf[:])
                    nc.vector.tensor_copy(sel16[:, 8:NW], wibf[:, 0:NW - 8])
                    # eq[p, j, w] = (ioWN[w] == sel16[j]); wsel[j] = sum_w eq * cpF[D+w]
                    eqw = sb.tile([128, NW, WIN], F32, tag="eqw", name="eqw")
                    s12 = sel16[:]
                    s12b = bass.AP(s12.tensor, s12.offset, [s12.ap[0], [1, NW], [0, WIN]])
                    iw = ioWNf[:]
                    iwb = bass.AP(iw.tensor, iw.offset, [iw.ap[0], [0, NW], [1, WIN]])
                    nc.vector.tensor_tensor(out=eqw[:], in0=iwb, in1=s12b, op=AL.is_equal)
                    cw = cpF[:, DIRECT:DIRECT + WIN]
                    cwb = bass.AP(cw.tensor, cw.offset, [cw.ap[0], [0, NW], [1, WIN]])
                    nc.vector.tensor_tensor(out=eqw[:], in0=eqw[:], in1=cwb, op=AL.mult)
                    nc.vector.tensor_reduce(out=wsel[:], in_=eqw[:], axis=AXX, op=AL.add)
                    nc.vector.tensor_copy(kidxF[:, DIRECT:COARSE_K], wsel[:])
                    kidxU = sb.tile([128, COARSE_K], U32, tag="kidxU", name="kidxU")
                    nc.vector.tensor_copy(kidxU[:], kidxF[:])

                    # ---- fine scores (fp16 K rows, fp32 accum) ----
                    kc = gp.tile([128, COARSE_K, R], F32, tag="gat", name="kc")
                    for c in range(COARSE_K):
                        nc.gpsimd.indirect_dma_start(
                            out=kc[:, c, :], out_offset=None, in_=K32,
                            in_offset=bass.IndirectOffsetOnAxis(
                                ap=kidxU[:, c:c + 1], axis=0))
                    s20 = sb.tile([128, COARSE_K], F32, tag="s20", name="s20")
                    jnk2 = sb.tile([128, R], F32, tag="jnk2", name="jnk2")
                    for c in range(COARSE_K):
                        nc.vector.scalar_tensor_tensor(
                            out=jnk2[:], in0=kc[:, c, :], scalar=1.0, in1=q16[m][:],
                            op0=AL.mult, op1=AL.mult,
                            accum_out=s20[:, c:c + 1])

                    # top-10 of 20
                    s20b = sb.tile([128, COARSE_K], F32, tag="s20b", name="s20b")
                    nc.vector.tensor_copy(s20b[:], s20[:])
                    f8a = sb.tile([128, 8], F32, tag="f8a", name="f8a")
                    nc.vector.max(out=f8a[:], in_=s20b[:])
                    nc.vector.match_replace(out=s20b[:], in_to_replace=f8a[:],
                                            in_values=s20b[:], imm_value=NEG)
                    f8b = sb.tile([128, 8], F32, tag="f8b", name="f8b")
                    nc.vector.max(out=f8b[:], in_=s20b[:])
                    fia = sb.tile([128, 8], U32, tag="fia", name="fia")
                    nc.vector.max_index(out=fia[:], in_max=f8a[:], in_values=s20[:])
                    fib = sb.tile([128, 8], U32, tag="fib", name="fib")
                    nc.vector.max_index(out=fib[:], in_max=f8b[:], in_values=s20[:])

                    sc10 = sb.tile([128, FINE_K], F32, tag="sc10", name="sc10")
                    nc.vector.tensor_copy(sc10[:, 0:8], f8a[:])
                    nc.vector.tensor_copy(sc10[:, 8:10], f8b[:, 0:2])
                    c10 = sb.tile([128, FINE_K], F32, tag="c10", name="c10")
                    fiaf = sb.tile([128, 8], F32, tag="fiaf", name="fiaf")
                    nc.vector.tensor_copy(fiaf[:], fia[:])
                    fibf = sb.tile([128, 8], F32, tag="fibf", name="fibf")
                    nc.vector.tensor_copy(fibf[:], fib[:])
                    nc.vector.tensor_copy(c10[:, 0:8], fiaf[:])
                    nc.vector.tensor_copy(c10[:, 8:10], fibf[:, 0:2])

                    # softmax over sc10 / sqrt(R)
                    wts = sb.tile([128, FINE_K], F32, tag="wts", name="wts")
                    nc.vector.tensor_scalar(wts[:], sc10[:], f8a[:, 0:1], None,
                                            op0=AL.subtract)
                    ex = sb.tile([128, FINE_K], F32, tag="ex", name="ex")
                    nc.scalar.activation(ex[:], wts[:], mybir.ActivationFunctionType.Exp,
                                         bias=0.0, scale=float(1.0 / np.sqrt(R)))
                    ssum = sb.tile([128, 1], F32, tag="ssum", name="ssum")
                    nc.vector.tensor_reduce(out=ssum[:], in_=ex[:], axis=AXX, op=AL.add)
                    rsum = sb.tile([128, 1], F32, tag="rsum", name="rsum")
                    nc.vector.reciprocal(rsum[:], ssum[:])
                    nc.vector.tensor_scalar_mul(wts[:], ex[:], rsum[:, 0:1])

                    # fine global idx = Kidx[c10]
                    eq10 = sb.tile([128, FINE_K, COARSE_K], F32, tag="eq10", name="eq10")
                    c10ap = c10[:]
                    c10b = bass.AP(c10ap.tensor, c10ap.offset,
                                   [c10ap.ap[0], [1, FINE_K], [0, COARSE_K]])
                    i20 = io20f[:]
                    i20b = bass.AP(i20.tensor, i20.offset,
                                   [i20.ap[0], [0, FINE_K], [1, COARSE_K]])
                    nc.vector.tensor_tensor(out=eq10[:], in0=i20b, in1=c10b,
                                            op=AL.is_equal)
                    kF = kidxF[:]
                    kFb = bass.AP(kF.tensor, kF.offset,
                                  [kF.ap[0], [0, FINE_K], [1, COARSE_K]])
                    nc.vector.tensor_tensor(out=eq10[:], in0=eq10[:], in1=kFb,
                                            op=AL.mult)
                    g10 = sb.tile([128, FINE_K], F32, tag="g10", name="g10")
                    nc.vector.tensor_reduce(out=g10[:], in_=eq10[:], axis=AXX, op=AL.add)
                    g10u = sb.tile([128, FINE_K], U32, tag="g10u", name="g10u")
                    nc.vector.tensor_copy(g10u[:], g10[:])

                    # ---- gather V rows (fp16), upconvert, weighted sum in fp32 ----
                    acc = sb.tile([128, D], F32, tag="acc", name="acc")
                    vg32 = sb.tile([128, D], F32, tag="vg32", name="vg32")
                    for h in range(2):
                        vg = gp.tile([128, FINE_K // 2, D], F16, tag="gat", name="vg")
                        for f in range(FINE_K // 2):
                            fi = h * (FINE_K // 2) + f
                            nc.gpsimd.indirect_dma_start(
                                out=vg[:, f, :], out_offset=None, in_=V16,
                                in_offset=bass.IndirectOffsetOnAxis(
                                    ap=g10u[:, fi:fi + 1], axis=0))
                        for f in range(FINE_K // 2):
                            fi = h * (FINE_K // 2) + f
                            nc.scalar.copy(vg32[:], vg[:, f, :])
                            if fi == 0:
                                nc.vector.tensor_scalar_mul(acc[:], vg32[:],
                                                            wts[:, 0:1])
                            else:
                                nc.vector.scalar_tensor_tensor(
                                    out=acc[:], in0=vg32[:],
                                    scalar=wts[:, fi:fi + 1], in1=acc[:],
                                    op0=AL.mult, op1=AL.add)

                    ost = sb.tile([128, D], F32, tag="ost", name="ost")
                    nc.scalar.copy(ost[:], acc[:])
                    nc.scalar.dma_start(out[msl, :], ost[:])


        for _rep in range(repeat):
            _emit_all()

    nc.compile()
    return nc


_BUILD_CACHE = {}


def _get_nc(n_chunks, m_tiles):
    key = (n_chunks, m_tiles)
    if key not in _BUILD_CACHE:
        _BUILD_CACHE[key] = build(n_chunks, m_tiles)
    return _BUILD_CACHE[key]


def _prep_inputs(x, W_router, W_enc, K_all, V_all, cores=8):
    """Host-side sharding/staging. Returns (in_maps, meta)."""
    B, S, Dx = x.shape
    ntok_total = B * S
    ntok = ntok_total // cores
    n_chunks = NK // CHUNK
    xf = np.ascontiguousarray(x.reshape(ntok_total, Dx).astype(np.float32))
    W32 = np.asarray(W_router, dtype=np.float32)
    # fp8 W chunk layout: W8L[n*128+p, ((t*2+i)*4? ...)] -- see build()
    W8 = np.clip(W32 * WSCALE, -240, 240).astype(ml_dtypes.float8_e4m3)
    # d = 256t + 128i + p ; col = CHUNK*n + c
    W8r = W8.reshape(PAIRS, 2, 128, n_chunks, CHUNK)       # (t, i, p, n, c)
    W8L = np.ascontiguousarray(
        W8r.transpose(3, 2, 0, 1, 4).reshape(n_chunks * 128, 8 * CHUNK))
    WT32 = np.ascontiguousarray(W32.T)
    K32 = np.ascontiguousarray(np.asarray(K_all, np.float32))
    V16 = np.ascontiguousarray(np.asarray(V_all, np.float32).astype(np.float16))
    Wenc32 = np.ascontiguousarray(np.asarray(W_enc, np.float32))
    in_maps = []
    for c in range(cores):
        sl = slice(c * ntok, (c + 1) * ntok)
        xs = xf[sl]
        xT = np.ascontiguousarray(xs.T)
        xT8 = np.ascontiguousarray(
            np.clip(xT * XSCALE, -240, 240).astype(ml_dtypes.float8_e4m3))
        in_maps.append(dict(
            xT8=xT8,
            xT32=xT,
            x32=np.ascontiguousarray(xs),
            W8L=W8L, WT32=WT32, K32=K32, V16=V16, Wenc=Wenc32,
        ))
    return in_maps, (B, S, Dx, ntok)


def kernel(x, W_router, W_enc, K_all, V_all):
    cores = 8
    in_maps, (B, S, Dx, ntok) = _prep_inputs(x, W_router, W_enc, K_all, V_all,
                                             cores)
    nc = _get_nc(NK // CHUNK, ntok // 128)
    res = run_bass_kernel_spmd(nc, in_maps, core_ids=list(range(cores)))
    outs = [res.results[c]["out"] for c in range(cores)]
    full = np.concatenate(outs, axis=0)
    return full.reshape(B, S, Dx).astype(np.float32)


if __name__ == "__main__":
    rng = np.random.default_rng(0)
    x = rng.standard_normal((2, 2048, D), dtype=np.float32)
    W = rng.standard_normal((D, NK), dtype=np.float32) * 0.02
    We = rng.standard_normal((D, R), dtype=np.float32) * 0.02
    K = rng.standard_normal((NK, R), dtype=np.float32) * 0.02
    V = rng.standard_normal((NK, D), dtype=np.float32) * 0.02
    y = kernel(x, W, We, K, V)
    print(y.shape, y.dtype)

